# revision 13
# baseline (speedup 1.0000x reference)
"""Trainium2 Bass kernel for nn_GATNet (2-layer GAT on two 100k-node KGs + TransE).

8-core SPMD design (one program, per-core tensor data):
  - dst-range sharding: core c owns nodes [c*N/8,(c+1)*N/8). Host sorts each
    graph's edges by dst, packs them into 128-edge groups per (128-node dst
    block, src&3 bank) with a data-derived static groups-per-bank count.
  - Dense h = x @ [Wcat|B] computed replicated into an HBM table
    hx[n] = [h(128)|es(2)|ed(2)|pad(60)] (192 f32, %256B rows for dma_gather),
    plus esd[n] = [es|ed] (4 f32).
  - Edge phase: per (4-block superblock, bank) chunk one multi-index
    dma_gather fetches h|es rows (bank = strided view hx[k::4]); one
    dma_gather per superblock fetches [es|ed] by LOCAL dst from a per-core
    esd_local table (built by a data-indexed indirect DMA, sidestepping SPMD).
    Segment softmax (no max-shift; |e|<0.6) aggregated via one-hot S matmuls
    into PSUM per dst block. No scatters.
  - AllGather exchanges x1 (transposed) and out1 between layers/cores.
  - TransE: triples bucketed by (h&3, t&3) so h/t rows come from bank-strided
    dma_gathers; rel rows gathered directly (R=1000 fits int16).
"""

import math

import numpy as np

NCORES = 8
NBANK = 4
SBW = 1          # blocks per superblock
ROWW = 192       # hx row width (f32) — 768B, %256
GB = 4           # one-hot groups per is_equal batch


# ---------------------------------------------------------------------------
# Host-side preprocessing
# ---------------------------------------------------------------------------

def _wrap16(flat):
    """dma_gather idx layout: idx j at [j%16, j//16], replicated to all 8
    16-partition groups.  flat: [..., L] -> [..., 128, L//16] int16."""
    a = np.asarray(flat, np.int16)
    L = a.shape[-1]
    w = a.reshape(*a.shape[:-1], L // 16, 16)
    w = np.swapaxes(w, -1, -2)                       # [..., 16, L//16]
    return np.broadcast_to(w[..., None, :, :],
                           (*a.shape[:-1], 8, 16, L // 16)).reshape(
                               *a.shape[:-1], 128, L // 16)


def _pack_edges(edges, n, ncores):
    src, dst = np.asarray(edges[0], np.int64), np.asarray(edges[1], np.int64)
    nc_nodes = n // ncores
    nblk = (nc_nodes + 127) // 128
    nsb = (nblk + SBW - 1) // SBW

    order = np.argsort(dst, kind="stable")
    s_src, s_dst = src[order], dst[order]
    bounds = np.searchsorted(s_dst, np.arange(ncores + 1) * nc_nodes)

    # counts per (core, block, bank) -> static GPBB
    gpbb = 0
    percore = []
    for c in range(ncores):
        lo, hi = bounds[c], bounds[c + 1]
        d = s_dst[lo:hi] - c * nc_nodes
        e_src = s_src[lo:hi]
        blk = d >> 7
        bank = e_src & 3
        key = blk * NBANK + bank
        cnt = np.bincount(key, minlength=nblk * NBANK)
        gpbb = max(gpbb, int(np.ceil(cnt.max() / 128)))
        percore.append((e_src, d, key, cnt))

    GPB = NBANK * gpbb
    G = nblk * GPB
    # slot arrays in (block, bank, j, slot) order first; reorder to chunk order later
    src_rows = np.zeros((ncores, G, 128), np.int64)
    dstloc = np.full((ncores, G, 128), 128.0, np.float32)
    dloc_i = np.zeros((ncores, G, 128), np.int64)      # local dst (for esd gather)
    for c in range(ncores):
        e_src, d, key, cnt = percore[c]
        korder = np.argsort(key, kind="stable")
        e_src, d, key = e_src[korder], d[korder], key[korder]
        starts = np.concatenate([[0], np.cumsum(cnt)])
        rank = np.arange(len(d)) - starts[key]
        blk, bank = key // NBANK, key % NBANK
        gidx = blk * GPB + bank * gpbb + (rank >> 7)
        slot = rank & 127
        src_rows[c, gidx, slot] = e_src >> 2
        dstloc[c, gidx, slot] = (d & 127).astype(np.float32)
        dloc_i[c, gidx, slot] = d

    # chunk-order metadata: chunk = (sb, bank), groups (bi, j)
    chunk_groups = []        # list per chunk of group ids (in block,bank,j space)
    sb_groups = []           # per sb: concatenated over banks
    for sb in range(nsb):
        b0 = sb * SBW
        nb = min(SBW, nblk - b0)
        sbg = []
        for bank in range(NBANK):
            ids = [(b0 + bi) * GPB + bank * gpbb + j
                   for bi in range(nb) for j in range(gpbb)]
            chunk_groups.append(ids)
            sbg.extend(ids)
        sb_groups.append(sbg)

    # gather idx arrays (wrapped int16), chunk-concatenated
    srcw = []
    for ids in chunk_groups:
        flat = src_rows[:, ids, :].reshape(ncores, -1)     # [C, len]
        srcw.append(_wrap16(flat))
    srcw = np.concatenate(srcw, axis=-1)                   # [C, 128, G*8]
    esdw = []
    for sbg in sb_groups:
        flat = dloc_i[:, sbg, :].reshape(ncores, -1)
        esdw.append(_wrap16(flat))
    esdw = np.concatenate(esdw, axis=-1)

    # dstloc in chunk-position order for S building
    pos_order = [g for ids in chunk_groups for g in ids]
    dstloc_pos = dstloc[:, pos_order, :]                   # [C, G, 128]
    dstloc_dev = np.ascontiguousarray(np.swapaxes(dstloc_pos, 1, 2))  # [C,128,G]

    meta = dict(GPBB=gpbb, GPB=GPB, G=G, NBLK=nblk, NSB=nsb,
                chunk_groups=chunk_groups, sb_groups=sb_groups)
    return srcw, esdw, dstloc_dev.astype(np.float32), meta


def _pack_triples(h, t, r, n, T, ncores, cht):
    """Bucket per-core triples by (h&3, t&3); pad buckets to 128 multiples.
    Returns wrapped idx arrays + slot->triple map."""
    h = np.asarray(h, np.int64); t = np.asarray(t, np.int64); r = np.asarray(r, np.int64)
    tpc = (T + ncores - 1) // ncores
    ngrp = 0
    binfo = []
    for c in range(ncores):
        hh = h[c * tpc: min((c + 1) * tpc, T)]
        tt = t[c * tpc: min((c + 1) * tpc, T)]
        key = (hh & 3) * 4 + (tt & 3)
        cnt = np.bincount(key, minlength=16)
        binfo.append((hh, tt, r[c * tpc: min((c + 1) * tpc, T)], key, cnt))
        ngrp = max(ngrp, int(np.ceil(cnt / 128).sum()))
    # static: groups per bucket NOT static; instead static TOTAL groups per core,
    # with per-bucket counts data-driven -> but gather instruction per bucket must
    # be static size. Use static per-bucket group count = max over cores+buckets.
    gpb = 0
    for _, _, _, key, cnt in binfo:
        gpb = max(gpb, int(np.ceil(cnt.max() / 128)))
    GT = 16 * gpb                                   # groups per core per graph
    hw_ = np.zeros((ncores, GT * 128), np.int64)
    tw_ = np.zeros((ncores, GT * 128), np.int64)
    rw_ = np.zeros((ncores, GT * 128), np.int64)
    smap = np.full((ncores, GT * 128), -1, np.int64)  # slot -> triple id (global)
    for c in range(ncores):
        hh, tt, rr, key, cnt = binfo[c]
        korder = np.argsort(key, kind="stable")
        starts = np.concatenate([[0], np.cumsum(cnt)])
        for b in range(16):
            sel = korder[starts[b]:starts[b + 1]]
            base = b * gpb * 128
            L = len(sel)
            hw_[c, base:base + L] = hh[sel] >> 2
            tw_[c, base:base + L] = tt[sel] >> 2
            rw_[c, base:base + L] = rr[sel]
            smap[c, base:base + L] = sel + c * tpc
    return (_wrap16(hw_.reshape(ncores, 16, gpb * 128)),
            _wrap16(tw_.reshape(ncores, 16, gpb * 128)),
            _wrap16(rw_.reshape(ncores, 16, gpb * 128)),
            smap, gpb)


def _preprocess(inputs, cfg):
    n = cfg["N"]
    ncores = NCORES
    nc_nodes = n // ncores

    W = np.asarray(inputs["W"], np.float32)
    a_s = np.asarray(inputs["a_src"], np.float32)
    a_d = np.asarray(inputs["a_dst"], np.float32)
    L, H, D, HD = W.shape

    wb = np.zeros((L, D, ROWW), np.float32)
    for l in range(L):
        wcat = W[l].transpose(1, 0, 2).reshape(D, H * HD)
        asd = np.zeros((D, 4), np.float32)
        for hh in range(H):
            asd[hh * HD:(hh + 1) * HD, hh] = a_s[l, hh]
            asd[hh * HD:(hh + 1) * HD, 2 + hh] = a_d[l, hh]
        wb[l, :, :132] = np.concatenate([wcat, wcat @ asd], axis=1)

    meta = {}
    eg = {}
    for g in ("sr", "tg"):
        srcw, esdw, dlocF, em = _pack_edges(inputs[f"edge_{g}"], n, ncores)
        eg[g] = (srcw, esdw, dlocF)
        meta[g] = em

    iota = np.tile(np.arange(128, dtype=np.float32), GB)[None, :].repeat(128, 0)
    ident = np.eye(128, dtype=np.float32)

    T = cfg["T"]
    CH_T = cfg["CH_T"]
    tr = {}
    for g in ("sr", "tg"):
        hw_, tw_, rw_, smap, gpbt = _pack_triples(
            inputs[f"h_list_{g}"], inputs[f"t_list_{g}"], inputs[f"r_list_{g}"],
            n, T, ncores, CH_T)
        tr[g] = (hw_, tw_, rw_)
        meta[f"smap_{g}"] = smap
        meta[f"gpbt_{g}"] = gpbt

    # selection: shard + pad to 128-multiples per core, bucket by &3
    S_CNT = cfg["S_CNT"]
    sel = {}
    for g, key in (("sr", "sr_data"), ("tg", "tg_data")):
        lst = np.asarray(inputs[key], np.int64)
        spc = (S_CNT + ncores - 1) // ncores
        sgrp = 0
        sinfo = []
        for c in range(ncores):
            ss = lst[c * spc: min((c + 1) * spc, S_CNT)]
            kk = ss & 3
            cnt = np.bincount(kk, minlength=4)
            sinfo.append((ss, kk, cnt))
            sgrp = max(sgrp, int(np.ceil(cnt.max() / 128)))
        GS = 4 * sgrp
        sw_ = np.zeros((ncores, GS * 128), np.int64)
        ssmap = np.full((ncores, GS * 128), -1, np.int64)
        for c in range(ncores):
            ss, kk, cnt = sinfo[c]
            korder = np.argsort(kk, kind="stable")
            starts = np.concatenate([[0], np.cumsum(cnt)])
            for b in range(4):
                selq = korder[starts[b]:starts[b + 1]]
                base = b * sgrp * 128
                sw_[c, base:base + len(selq)] = ss[selq] >> 2
                ssmap[c, base:base + len(selq)] = selq + c * spc
        sel[g] = _wrap16(sw_.reshape(ncores, 4, sgrp * 128))
        meta[f"ssmap_{g}"] = ssmap
        meta[f"sgrp_{g}"] = sgrp

    # per-core esd-local fetch index: partition p -> global row c*nc + p*rows_pp
    nloc_pad = ((nc_nodes + 127) // 128) * 128
    rows_pp = nloc_pad // 128
    meta["rows_pp"] = rows_pp
    meta["nloc_pad"] = nloc_pad

    in_maps = []
    x0 = {g: np.ascontiguousarray(np.asarray(inputs[f"ent_{g}"], np.float32).T)
          for g in ("sr", "tg")}
    rel = {g: np.asarray(inputs[f"rel_{g}"], np.float32) for g in ("sr", "tg")}

    for c in range(ncores):
        m = {"wb0": wb[0], "wb1": wb[1], "iota": iota, "ident": ident,
             "lfetch": (c * nc_nodes + np.arange(128, dtype=np.int32)[:, None] * rows_pp)}
        for g in ("sr", "tg"):
            srcw, esdw, dlocF = eg[g]
            m[f"x0T_{g}"] = x0[g]
            m[f"rel_{g}"] = rel[g]
            m[f"srcw_{g}"] = srcw[c]
            m[f"esdw_{g}"] = esdw[c]
            m[f"dlocF_{g}"] = dlocF[c]
            hw_, tw_, rw_ = tr[g]
            m[f"hw_{g}"] = np.ascontiguousarray(hw_[c].transpose(1, 0, 2).reshape(128, -1))
            m[f"tw_{g}"] = np.ascontiguousarray(tw_[c].transpose(1, 0, 2).reshape(128, -1))
            m[f"rw_{g}"] = np.ascontiguousarray(rw_[c].transpose(1, 0, 2).reshape(128, -1))
            m[f"selw_{g}"] = np.ascontiguousarray(sel[g][c].transpose(1, 0, 2).reshape(128, -1))
        in_maps.append(m)
    return in_maps, meta


# ---------------------------------------------------------------------------
# Bass program
# ---------------------------------------------------------------------------

def _build_program(cfg, meta):
    import concourse.bacc as bacc
    import concourse.tile as tile
    from concourse import mybir, library_config
    from concourse.bass import IndirectOffsetOnAxis

    f32 = mybir.dt.float32
    i32 = mybir.dt.int32
    i16 = mybir.dt.int16
    AF = mybir.ActivationFunctionType
    OP = mybir.AluOpType

    N = cfg["N"]
    NPAD = ((N + 127) // 128) * 128 + 128
    NC = N // NCORES
    ALPHA = 0.2
    RG = [list(range(NCORES))]
    ST = cfg["ST"]
    CH_T = cfg["CH_T"]
    nloc_pad = meta["nloc_pad"]

    nc = bacc.Bacc(None, num_devices=NCORES)

    inp = {}
    for name, shape, dt in [
        ("wb0", [128, ROWW], f32), ("wb1", [128, ROWW], f32),
        ("iota", [128, GB * 128], f32), ("ident", [128, 128], f32),
        ("lfetch", [128, 1], i32),
    ]:
        inp[name] = nc.dram_tensor(name, shape, dt, kind="ExternalInput")
    for g in ("sr", "tg"):
        em = meta[g]
        G = em["G"]
        gpbt = meta[f"gpbt_{g}"]
        sgrp = meta[f"sgrp_{g}"]
        inp[f"x0T_{g}"] = nc.dram_tensor(f"x0T_{g}", [128, N], f32, kind="ExternalInput")
        inp[f"rel_{g}"] = nc.dram_tensor(f"rel_{g}", [cfg["R"], 128], f32, kind="ExternalInput")
        inp[f"srcw_{g}"] = nc.dram_tensor(f"srcw_{g}", [128, G * 8], i16, kind="ExternalInput")
        inp[f"esdw_{g}"] = nc.dram_tensor(f"esdw_{g}", [128, G * 8], i16, kind="ExternalInput")
        inp[f"dlocF_{g}"] = nc.dram_tensor(f"dlocF_{g}", [128, G], f32, kind="ExternalInput")
        for k in ("hw", "tw", "rw"):
            inp[f"{k}_{g}"] = nc.dram_tensor(f"{k}_{g}", [128, 16 * gpbt * 8], i16, kind="ExternalInput")
        inp[f"selw_{g}"] = nc.dram_tensor(f"selw_{g}", [128, 4 * sgrp * 8], i16, kind="ExternalInput")

    out_sel = {g: nc.dram_tensor(f"selout_{g}", [4 * meta[f"sgrp_{g}"] * 128, 128], f32,
                                 kind="ExternalOutput") for g in ("sr", "tg")}
    PADT = {g: 16 * meta[f"gpbt_{g}"] * 128 for g in ("sr", "tg")}
    out_tr = nc.dram_tensor("transe", [PADT["sr"] + PADT["tg"], 128], f32, kind="ExternalOutput")

    hx = {(g, l): nc.dram_tensor(f"hx_{g}{l}", [NPAD, ROWW], f32)
          for g in ("sr", "tg") for l in (0, 1)}
    esdg = {(g, l): nc.dram_tensor(f"esdg_{g}{l}", [NPAD, 4], f32)
            for g in ("sr", "tg") for l in (0, 1)}
    esdl = {(g, l): nc.dram_tensor(f"esdl_{g}{l}", [nloc_pad, 64], f32)
            for g in ("sr", "tg") for l in (0, 1)}
    x1T_sh = {g: nc.dram_tensor(f"x1Tsh_{g}", [128, NC], f32) for g in ("sr", "tg")}
    x1T_all = {g: nc.dram_tensor(f"x1Tall_{g}", [128 * NCORES, NC], f32, addr_space="Shared")
               for g in ("sr", "tg")}
    o1_sh = {g: nc.dram_tensor(f"o1sh_{g}", [NC, 128], f32) for g in ("sr", "tg")}
    o1_all = {g: nc.dram_tensor(f"o1all_{g}", [N, 128], f32, addr_space="Shared")
              for g in ("sr", "tg")}

    with tile.TileContext(nc) as tc:
        import contextlib
        ctx = contextlib.ExitStack()
        with ctx:
            consts = ctx.enter_context(tc.tile_pool(name="consts", bufs=1))
            p1x = ctx.enter_context(tc.tile_pool(name="p1x", bufs=3))
            p1h = ctx.enter_context(tc.tile_pool(name="p1h", bufs=3))
            psum1 = ctx.enter_context(tc.tile_pool(name="psum1", bufs=2, space="PSUM"))
            epsum = ctx.enter_context(tc.tile_pool(name="epsum", bufs=4, space="PSUM"))
            tpsum = ctx.enter_context(tc.tile_pool(name="tpsum", bufs=2, space="PSUM"))
            eidx = ctx.enter_context(tc.tile_pool(name="eidx", bufs=2))
            egath = ctx.enter_context(tc.tile_pool(name="egath", bufs=2))
            emsx = ctx.enter_context(tc.tile_pool(name="emsx", bufs=2))
            eatt = ctx.enter_context(tc.tile_pool(name="eatt", bufs=2))
            esel = ctx.enter_context(tc.tile_pool(name="esel", bufs=2))
            efin = ctx.enter_context(tc.tile_pool(name="efin", bufs=2))
            trp = ctx.enter_context(tc.tile_pool(name="trp", bufs=2))

            from concourse import library_config as LC
            nc.gpsimd.load_library(LC.mlp)

            MAXI = 1024

            def gather_capped(out3, src_ap, idxt, ni, elem, elem_step=None):
                # out3: [128, ni//128, elem] view; idxt: [128, ni//16] tile view
                for i0 in range(0, ni, MAXI):
                    L = min(MAXI, ni - i0)
                    nc.gpsimd.dma_gather(
                        out3[:, i0 // 128:(i0 + L) // 128, :], src_ap,
                        idxt[:, i0 // 16:(i0 + L) // 16], L, L, elem,
                        elem_step=elem_step)

            wbt = {}
            for l, name in ((0, "wb0"), (1, "wb1")):
                t = consts.tile([128, ROWW], f32, tag=name)
                nc.sync.dma_start(out=t[:], in_=inp[name][:, :])
                wbt[l] = t
            iota_t = consts.tile([128, GB * 128], f32, tag="iota")
            nc.sync.dma_start(out=iota_t[:], in_=inp["iota"][:, :])
            ident_t = consts.tile([128, 128], f32, tag="ident")
            nc.sync.dma_start(out=ident_t[:], in_=inp["ident"][:, :])
            lf_t = consts.tile([128, 1], i32, tag="lfetch")
            nc.sync.dma_start(out=lf_t[:], in_=inp["lfetch"][:, :])

            # ---------------- phase 1 ----------------
            def phase1(g, l):
                table = hx[(g, l)]
                esdt = esdg[(g, l)]
                for r in range(NCORES):
                    base = r * NC
                    for st0 in range(0, NC, ST):
                        w = min(ST, NC - st0)
                        nsub = (w + 127) // 128
                        xt = p1x.tile([128, ST], f32, tag="xt")
                        if l == 0:
                            nc.sync.dma_start(out=xt[:, :w],
                                              in_=inp[f"x0T_{g}"][:, base + st0: base + st0 + w])
                        else:
                            nc.sync.dma_start(out=xt[:, :w],
                                              in_=x1T_all[g][r * 128:(r + 1) * 128, st0: st0 + w])
                        hxsb = p1h.tile([128, (ST // 128) * ROWW], f32, tag="hxsb")
                        esb = p1h.tile([128, (ST // 128) * 4], f32, tag="esb")
                        for s in range(nsub):
                            ww = min(128, w - s * 128)
                            ps = psum1.tile([128, ROWW], f32, tag="ps1")
                            nc.tensor.matmul(ps[:ww, :], lhsT=xt[:, s * 128: s * 128 + ww],
                                             rhs=wbt[l][:], start=True, stop=True)
                            nc.vector.tensor_copy(hxsb[:ww, s * ROWW:(s + 1) * ROWW], ps[:ww, :])
                            nc.vector.tensor_copy(esb[:ww, s * 4:(s + 1) * 4], ps[:ww, 128:132])
                        n0 = base + st0
                        if w % 128 == 0:
                            nc.sync.dma_start(
                                out=table[n0: n0 + w, :].rearrange("(s p) c -> p s c", p=128),
                                in_=hxsb[:, : nsub * ROWW].rearrange("p (s c) -> p s c", c=ROWW))
                            nc.sync.dma_start(
                                out=esdt[n0: n0 + w, :].rearrange("(s p) c -> p s c", p=128),
                                in_=esb[:, : nsub * 4].rearrange("p (s c) -> p s c", c=4))
                        else:
                            for s in range(nsub):
                                ww = min(128, w - s * 128)
                                nc.sync.dma_start(
                                    out=table[n0 + s * 128: n0 + s * 128 + ww, :],
                                    in_=hxsb[:ww, s * ROWW:(s + 1) * ROWW])
                                nc.sync.dma_start(
                                    out=esdt[n0 + s * 128: n0 + s * 128 + ww, :],
                                    in_=esb[:ww, s * 4:(s + 1) * 4])

            # ---------------- edge phase ----------------
            def edge_phase(g, l):
                em = meta[g]
                G, GPB, GPBB, NBLK, NSB = em["G"], em["GPB"], em["GPBB"], em["NBLK"], em["NSB"]
                table = hx[(g, l)]

                # build per-core esd_local
                esdsb = eidx.tile([128, meta["rows_pp"] * 4], f32, tag="esdsb")
                nc.gpsimd.indirect_dma_start(
                    out=esdsb[:], out_offset=None, in_=esdg[(g, l)][:, :],
                    in_offset=IndirectOffsetOnAxis(ap=lf_t[:, :], axis=0))
                nc.sync.dma_start(
                    out=esdl[(g, l)][:, 0:4].rearrange("(p j) c -> p j c", p=128),
                    in_=esdsb[:].rearrange("p (j c) -> p j c", c=4))

                dlsb = eidx.tile([128, G], f32, tag="dlsb")
                nc.sync.dma_start(out=dlsb[:], in_=inp[f"dlocF_{g}"][:, :])

                gpos = 0       # chunk-order group position
                scol = 0       # src idx col offset (G*8 total)
                ecol = 0
                for sb in range(NSB):
                    b0 = sb * SBW
                    nb = min(SBW, NBLK - b0)
                    sbgroups = nb * GPB
                    sbedges = sbgroups * 128
                    # esd gather for whole superblock
                    eidx_t = eidx.tile([128, (SBW * NBANK * GPBB * 128) // 16], i16, tag="eidx_t")
                    nc.sync.dma_start(out=eidx_t[:, : sbedges // 16],
                                      in_=inp[f"esdw_{g}"][:, ecol: ecol + sbedges // 16])
                    ecol += sbedges // 16
                    esdt = egath.tile([128, SBW * NBANK * GPBB * 64], f32, tag="esdt")
                    gather_capped(
                        esdt[:].rearrange("p (a b) -> p a b", b=64)[:, : sbgroups, :],
                        esdl[(g, l)][:, :], eidx_t[:, : sbedges // 16],
                        sbedges, 64)

                    psb = [None] * nb
                    for bank in range(NBANK):
                        cg = nb * GPBB                     # groups this chunk
                        ce = cg * 128
                        sidx_t = eidx.tile([128, (SBW * GPBB * 128) // 16], i16, tag="sidx_t")
                        nc.sync.dma_start(out=sidx_t[:, : ce // 16],
                                          in_=inp[f"srcw_{g}"][:, scol: scol + ce // 16])
                        scol += ce // 16
                        hg = egath.tile([128, SBW * GPBB * ROWW], f32, tag="hg")
                        gather_capped(
                            hg[:].rearrange("p (a b) -> p a b", b=ROWW)[:, : cg, :],
                            table[bank::NBANK, :], sidx_t[:, : ce // 16],
                            ce, ROWW, elem_step=NBANK * ROWW)

                        # e_att for this chunk
                        hg3 = hg[:].rearrange("p (a b) -> p a b", b=ROWW)
                        # esd slice for this bank: groups bank*cg .. (bank+1)*cg? NO:
                        # esd order is chunk-concatenated: bank-major within sb
                        ed3 = esdt[:].rearrange("p (a b) -> p a b", b=64)[:, bank * cg:(bank + 1) * cg, :]
                        msx = emsx.tile([128, SBW * GPBB * 130], f32, tag="msx")
                        ms3 = msx[:].rearrange("p (a b) -> p a b", b=130)
                        e1 = eatt.tile([128, SBW * GPBB * 2], f32, tag="e1")
                        e2 = eatt.tile([128, SBW * GPBB * 2], f32, tag="e2")
                        e13 = e1[:].rearrange("p (a b) -> p a b", b=2)
                        e23 = e2[:].rearrange("p (a b) -> p a b", b=2)
                        nc.vector.tensor_add(e13[:, :cg, :], hg3[:, :cg, 128:130], ed3[:, :, 2:4])
                        nc.vector.tensor_scalar_mul(e23[:, :cg, :], e13[:, :cg, :], ALPHA)
                        nc.vector.tensor_tensor(e23[:, :cg, :], e13[:, :cg, :], e23[:, :cg, :], op=OP.max)
                        nc.scalar.activation(ms3[:, :cg, 128:130], e23[:, :cg, :], AF.Exp)

                        S4 = None
                        for q in range(cg):           # group within chunk
                            if q % GB == 0:
                                jw = min(GB, cg - q)
                                S4 = esel.tile([128, GB * 128], f32, tag="S4")
                                nc.vector.tensor_tensor(
                                    S4[:, : jw * 128].rearrange("p (a b) -> p a b", b=128),
                                    dlsb[:, gpos + q: gpos + q + jw].to_broadcast([128, jw, 128]),
                                    iota_t[:, : jw * 128].rearrange("p (a b) -> p a b", b=128),
                                    op=OP.is_equal)
                            S = S4[:, (q % GB) * 128:(q % GB + 1) * 128]
                            nc.vector.tensor_scalar_mul(
                                msx[:, q * 130: q * 130 + 64],
                                hg[:, q * ROWW: q * ROWW + 64],
                                msx[:, q * 130 + 128: q * 130 + 129])
                            nc.vector.tensor_scalar_mul(
                                msx[:, q * 130 + 64: q * 130 + 128],
                                hg[:, q * ROWW + 64: q * ROWW + 128],
                                msx[:, q * 130 + 129: q * 130 + 130])
                            bi, j = divmod(q, GPBB)
                            if bank == 0 and j == 0:
                                psb[bi] = epsum.tile([128, 130], f32, tag="eps", name=f"eps_{b0+bi}")
                            nc.tensor.matmul(psb[bi][:], lhsT=S,
                                             rhs=msx[:, q * 130:(q + 1) * 130],
                                             start=(bank == 0 and j == 0),
                                             stop=(bank == NBANK - 1 and j == GPBB - 1))
                        gpos += cg
                    for bi in range(nb):
                        finalize(g, l, b0 + bi, psb[bi])

            def finalize(g, l, b, ps):
                wlen = min(128, NC - b * 128)
                den = efin.tile([128, 2], f32, tag="den")
                nc.vector.tensor_scalar_add(den[:], ps[:, 128:130], 1e-16)
                rec = efin.tile([128, 2], f32, tag="rec")
                nc.vector.reciprocal(rec[:], den[:])
                outf = efin.tile([128, 128], f32, tag="outf")
                nc.vector.tensor_scalar_mul(outf[:, 0:64], ps[:, 0:64], rec[:, 0:1])
                nc.vector.tensor_scalar_mul(outf[:, 64:128], ps[:, 64:128], rec[:, 1:2])
                if l == 0:
                    from concourse import mybir as mb
                    mask = efin.tile([128, 128], mb.dt.uint8, tag="mask")
                    nc.vector.tensor_scalar(mask[:], outf[:], 0.0, None, op0=OP.is_gt)
                    ex2 = efin.tile([128, 128], f32, tag="ex2")
                    nc.scalar.activation(ex2[:], outf[:], AF.Exp)
                    xf = efin.tile([128, 128], f32, tag="xf")
                    nc.vector.tensor_scalar_add(xf[:], ex2[:], -1.0)
                    nc.vector.copy_predicated(xf[:], mask[:], outf[:])
                    tp = tpsum.tile([128, 128], f32, tag="tp")
                    nc.tensor.transpose(tp[:], xf[:], ident_t[:])
                    xT = efin.tile([128, 128], f32, tag="xT")
                    nc.vector.tensor_copy(xT[:], tp[:])
                    nc.sync.dma_start(out=x1T_sh[g][:, b * 128: b * 128 + wlen],
                                      in_=xT[:, :wlen])
                else:
                    nc.sync.dma_start(out=o1_sh[g][b * 128: b * 128 + wlen, :],
                                      in_=outf[:wlen, :])

            # ---------------- transe + selection ----------------
            def transe(g, goff):
                gpbt = meta[f"gpbt_{g}"]
                ni = gpbt * 128
                for b in range(16):
                    ht = trp.tile([128, ni // 16], i16, tag="ht")
                    nc.sync.dma_start(out=ht[:], in_=inp[f"hw_{g}"][:, b * ni // 16:(b + 1) * ni // 16])
                    tt = trp.tile([128, ni // 16], i16, tag="tt")
                    nc.sync.dma_start(out=tt[:], in_=inp[f"tw_{g}"][:, b * ni // 16:(b + 1) * ni // 16])
                    rt = trp.tile([128, ni // 16], i16, tag="rt")
                    nc.sync.dma_start(out=rt[:], in_=inp[f"rw_{g}"][:, b * ni // 16:(b + 1) * ni // 16])
                    hb, tb = b // 4, b % 4
                    hrow = trp.tile([128, gpbt, 128], f32, tag="hrow")
                    gather_capped(hrow[:], o1_all[g][hb::NBANK, :], ht[:],
                                  ni, 128, elem_step=NBANK * 128)
                    trow = trp.tile([128, gpbt, 128], f32, tag="trow")
                    gather_capped(trow[:], o1_all[g][tb::NBANK, :], tt[:],
                                  ni, 128, elem_step=NBANK * 128)
                    rrow = trp.tile([128, gpbt, 128], f32, tag="rrow")
                    gather_capped(rrow[:], inp[f"rel_{g}"][:, :], rt[:],
                                  ni, 128)
                    sc = trp.tile([128, gpbt, 128], f32, tag="sc")
                    nc.vector.tensor_add(sc[:], hrow[:], rrow[:])
                    nc.vector.tensor_sub(sc[:], sc[:], trow[:])
                    # slot i = b*ni + j*128 + p  (j = idx//128 within bucket)
                    dst = out_tr[goff + b * ni: goff + (b + 1) * ni, :]
                    nc.sync.dma_start(
                        out=dst.rearrange("(j p) f -> p j f", p=128),
                        in_=sc[:])

            def selection(g):
                sgrp = meta[f"sgrp_{g}"]
                ni = sgrp * 128
                for b in range(4):
                    st_ = trp.tile([128, ni // 16], i16, tag="st_")
                    nc.sync.dma_start(out=st_[:], in_=inp[f"selw_{g}"][:, b * ni // 16:(b + 1) * ni // 16])
                    srow = trp.tile([128, sgrp, 128], f32, tag="srow")
                    gather_capped(srow[:], o1_all[g][b::NBANK, :], st_[:],
                                  ni, 128, elem_step=NBANK * 128)
                    dst = out_sel[g][b * ni:(b + 1) * ni, :]
                    nc.sync.dma_start(out=dst.rearrange("(j p) f -> p j f", p=128),
                                      in_=srow[:])

            # ---------------- emission ----------------
            import os
            _PH = int(os.environ.get("GAT_PHASES", "4"))
            for g in ("sr", "tg"):
                phase1(g, 0)
            for g in ("sr", "tg"):
                edge_phase(g, 0)
                from concourse import mybir as mb
                nc.gpsimd.collective_compute(
                    "AllGather", mb.AluOpType.bypass, replica_groups=RG,
                    ins=[x1T_sh[g][:, :]], outs=[x1T_all[g][:, :]])
            if _PH >= 2:
                for g in ("sr", "tg"):
                    phase1(g, 1)
                for g in ("sr", "tg"):
                    edge_phase(g, 1)
                    from concourse import mybir as mb
                    nc.gpsimd.collective_compute(
                        "AllGather", mb.AluOpType.bypass, replica_groups=RG,
                        ins=[o1_sh[g][:, :]], outs=[o1_all[g][:, :]])
            if _PH >= 3:
                transe("sr", 0)
                transe("tg", PADT["sr"])
            if _PH >= 4:
                selection("sr")
                selection("tg")

    nc.compile()
    return nc


# ---------------------------------------------------------------------------
# Entry point
# ---------------------------------------------------------------------------

def _default_cfg(inputs):
    n = int(np.asarray(inputs["ent_sr"]).shape[0])
    return {
        "N": n,
        "R": int(np.asarray(inputs["rel_sr"]).shape[0]),
        "T": int(np.asarray(inputs["h_list_sr"]).shape[0]),
        "S_CNT": int(np.asarray(inputs["sr_data"]).shape[0]),
        "CH_T": 8,
        "ST": 1024,
    }


def _assemble_outputs(inputs, results, meta, cfg):
    S_CNT = cfg["S_CNT"]
    T = cfg["T"]

    sel_out = {}
    for g in ("sr", "tg"):
        ssmap = meta[f"ssmap_{g}"]
        out = np.zeros((S_CNT, 128), np.float32)
        for c in range(NCORES):
            m = ssmap[c]
            v = m >= 0
            out[m[v]] = results[c][f"selout_{g}"][v]
        sel_out[g] = out

    score = {}
    off = 0
    for g in ("sr", "tg"):
        padt = 16 * meta[f"gpbt_{g}"] * 128
        smap = meta[f"smap_{g}"]
        out = np.zeros((T, 128), np.float32)
        for c in range(NCORES):
            m = smap[c]
            v = m >= 0
            out[m[v]] = results[c]["transe"][off: off + padt][v]
        score[g] = out
        off += padt

    transe = np.concatenate([score["sr"], score["tg"]], axis=0)
    return sel_out["sr"], sel_out["tg"], transe


def kernel(**inputs):
    from concourse.bass_utils import run_bass_kernel_spmd

    cfg = _default_cfg(inputs)
    in_maps, meta = _preprocess(inputs, cfg)
    nc = _build_program(cfg, meta)
    res = run_bass_kernel_spmd(nc, in_maps, core_ids=list(range(NCORES)))
    return _assemble_outputs(inputs, res.results, meta, cfg)
